# revision 39
# baseline (speedup 1.0000x reference)
"""Trainium2 Bass kernel for nn_ChannelMerger.

Computation (per batch b):
    emb   = fourier_emb(positions[b])            # [C, D]   D=288
    scores= emb @ heads.T                        # [C, O]   O=270 (kept transposed)
    w     = softmax(scores + mask_offset, axis=C)
    out[b]= (w.T @ meg[b])                       # [O, T]

Sharding: data-parallel over batch B=32 across 8 cores (4 batches/core).
heads + fourier constants replicated.  Softmax normalization is folded
into the PSUM->SBUF evacuation of the final matmul (scale by 1/sum_exp
per output row).

I/O in bf16: meg is cast f32->bf16 on the HOST (the big matmul consumed
bf16 anyway), and the output is stored bf16 and upcast f32 on the host.
This halves both directions of HBM traffic, which the f32 baseline
trace showed was the bottleneck (all 16 DMA engines ~77% busy).

Fourier embedding on device:
    loc[d, c] = x_c*px[d] + y_c*py[d] + (margin*(px+py)[d] + 2*pi*phase[d])
  computed as an outer product on the DVE (2 tensor_scalar ops with
  per-partition scalar columns px/py/const; x and y pre-replicated to 96
  rows on the host) — keeps the PE free for the big matmul.  phase =
  0.25 turns for the cos half (d<144), 0 for the sin half.  Then
  t = loc/(2*pi); r = round(t) via the +-1.5*2^23 magic trick;
  emb = Sin(2*pi*(t - r)), argument in [-pi,pi].

Perf notes (HW-measured on these cores):
  - The HAM clock manager down-clocks the PE 2.4->1.2 GHz on idle gaps
    and odd matmul geometries (K<96 row groups, M=1 outputs), with
    ~3.4us hysteresis windows; a t-major mapping of the big matmul
    (stationary=meg, N=272) was tried and is theoretically 30% cheaper,
    but its tighter rhythm + evac stalls kept tripping the down-clock
    and lost to this o-partition mapping.  Keep every matmul K=96,
    M>=96, and keep the PE FED.
  - Back-to-back N=512 bf16 matmuls sustain 216 ns (1 col/cycle) with
    LDWEIGHTS fully hidden.
  - O padded to 272 on the weights path so bf16 moving rows are
    8-byte-aligned (odd-word rows stream ~1.35 cyc/col).
  - scores matmul in bf16 (emb + heads bf16): fp32r streams 2 cyc/col.
  - weights for batch b+1 are emitted BEFORE batch b's big matmul so the
    cheap critical-path ops sit ahead of bulk evacuation work in every
    engine's FIFO.
  - meg tiles are prefetched one t-tile ahead on the SWDGE queue.
"""

import math

import ml_dtypes
import numpy as np

import concourse.bacc as bacc
import concourse.bass as bass
import concourse.mybir as mybir
from concourse.bass_utils import run_bass_kernel_spmd
from concourse.tile import TileContext

# Problem shape (hardcoded per contract)
B, C, T = 32, 273, 4096
O, D = 270, 288
OP = 272           # O padded so bf16 moving rows are 8-byte aligned (544B)
NF = 12            # fourier freqs per axis (sqrt(D/2))
MARGIN = 0.1
NCORES = 8
BPC = B // NCORES  # batches per core

TT = 1024          # T tile (columns of the big matmul kept in SBUF at once)
NT = T // TT
MM_N = 512         # moving free dim per matmul / one PSUM bank of fp32

KC = 96            # uniform contraction chunk (full PE row groups)
# (start, n_zero_weight_rows) for the C (channel) contraction chunks
C_CHUNKS = [(0, 0), (96, 0), (C - KC, 2 * KC - (C - 96))]    # 177: 15 dup rows
D_CHUNKS = [0, 96, 192]                                      # D = 3*96 exact
O_CHUNKS = [0, 128, O - 128]                                 # out row starts, M=128
KPAD = 96          # loc matmul K padding (K<96 geometries hurt the PE clock)

MAGIC = 1.5 * 2.0**23       # fp32 round-to-nearest-integer magic constant
TWO_PI = 2.0 * math.pi
NEG_BIG = -1.0e30           # stands in for -inf on masked channels
CP = C + 1                  # C padded to even for fp32r matmul free-dim rules

F32 = mybir.dt.float32
F32R = mybir.dt.float32r
BF16 = mybir.dt.bfloat16
BF_NP = ml_dtypes.bfloat16

_CACHE = {}
LAST_RESULTS = None         # BassKernelResults of the most recent run (for test.py)


def _fourier_consts():
    """p3t [KPAD, D]: rows px, py, additive const, then zero padding."""
    p = (2.0 * math.pi / (1.0 + 2.0 * MARGIN)) * np.arange(NF, dtype=np.float64)
    dd = np.arange(D) % (NF * NF)
    fx, fy = dd // NF, dd % NF
    px, py = p[fx], p[fy]
    phase = np.where(np.arange(D) < NF * NF, 0.25, 0.0)  # cos half first
    const = MARGIN * (px + py) + TWO_PI * phase
    out = np.zeros((KPAD, D), np.float32)
    out[0], out[1], out[2] = px, py, const
    return out


def _build_program():
    nc = bacc.Bacc(
        trn_type="TRN2",
        target_bir_lowering=False,
        debug=False,
        dynamic_dma_scratch_size=32768,
    )

    meg = nc.dram_tensor("meg", [BPC, C, T], BF16, kind="ExternalInput").ap()
    # f32 constants in two tensors: fc0 = what batch 0's loc needs (small,
    # lands first): x_bc|y_bc (positions replicated to 96 rows) + px|py|const
    # per-partition columns per d-chunk; fconst = batches 1..3 + mask offsets
    F0COLS = 2 * CP + 3 * len(D_CHUNKS)
    fconst0 = nc.dram_tensor("fconst0", [KC, F0COLS], F32, kind="ExternalInput").ap()
    FCOLS = (BPC - 1) * 2 * CP + BPC * len(C_CHUNKS)
    fconst = nc.dram_tensor("fconst", [KC, FCOLS], F32, kind="ExternalInput").ap()
    # heads chunks side by side: [KC, 3*OP] bf16, one DMA
    headsT = nc.dram_tensor(
        "headsT", [KC, len(D_CHUNKS) * OP], BF16, kind="ExternalInput"
    ).ap()
    out = nc.dram_tensor("out", [BPC, O, T], BF16, kind="ExternalOutput").ap()

    with TileContext(nc) as tc:
        with (
            tc.tile_pool(name="singles", bufs=1) as singles,
            tc.tile_pool(name="w", bufs=2) as wp,
            tc.tile_pool(name="megp", bufs=6) as megp,
            tc.tile_pool(name="outp", bufs=2) as outp,
            tc.tile_pool(name="psmall", bufs=3, space="PSUM") as psmall,
            tc.tile_pool(name="psbig", bufs=5, space="PSUM") as psbig,
        ):
            # ---- replicated constants (3 DMAs; fc0 lands first) ----
            fc0 = singles.tile([KC, F0COLS], F32, name="fconst0_sb")
            nc.sync.dma_start(out=fc0, in_=fconst0)
            fc = singles.tile([KC, FCOLS], F32, name="fconst_sb")
            nc.sync.dma_start(out=fc, in_=fconst)
            xys = [(fc0[:, 0:CP], fc0[:, CP : 2 * CP])] + [
                (
                    fc[:, (b - 1) * 2 * CP : (b - 1) * 2 * CP + CP],
                    fc[:, (b - 1) * 2 * CP + CP : b * 2 * CP],
                )
                for b in range(1, BPC)
            ]
            pxc = fc0[:, 2 * CP : 2 * CP + 3]
            pyc = fc0[:, 2 * CP + 3 : 2 * CP + 6]
            cnc = fc0[:, 2 * CP + 6 : 2 * CP + 9]
            offs_all = fc[:, (BPC - 1) * 2 * CP :]
            heads_sb = singles.tile([KC, len(D_CHUNKS) * OP], BF16, name="heads_sb")
            nc.sync.dma_start(out=heads_sb, in_=headsT)
            headsT_sb = [
                heads_sb[:, k * OP : (k + 1) * OP] for k in range(len(D_CHUNKS))
            ]
            ones_sb = singles.tile([KC, 1], BF16, name="ones_sb")
            nc.vector.memset(ones_sb, 1.0)

            embT = {}
            expT = {}
            inv = {}
            megs = {}

            def load_tile(b, th):
                """issue the 3 meg-chunk DMAs for one T tile (SWDGE queue)."""
                t0 = th * TT
                for j, (c0, _) in enumerate(C_CHUNKS):
                    mg = megp.tile(
                        [KC, TT], BF16, name=f"meg_b{b}t{th}j{j}", tag=f"meg{j}"
                    )
                    nc.gpsimd.dma_start(
                        out=mg, in_=meg[b, c0 : c0 + KC, t0 : t0 + TT]
                    )
                    megs[(b, th, j)] = mg

            def compute_wA(b):
                """emb + scores + exp for batch b (ACT: Sin then Exp).
                loc[d,c] = x_c*px[d] + y_c*py[d] + const[d] is an outer
                product: 2 DVE ops with per-partition scalar APs (keeps the
                PE free for the big matmul)."""
                xb, yb = xys[b]
                for k, d0 in enumerate(D_CHUNKS):
                    t1_ = wp.tile([KC, CP], F32, name=f"t1_b{b}k{k}", tag="t1", bufs=3)
                    nc.vector.tensor_scalar(
                        t1_,
                        xb,
                        pxc[:, k : k + 1],
                        cnc[:, k : k + 1],
                        op0=mybir.AluOpType.mult,
                        op1=mybir.AluOpType.add,
                    )
                    locp = wp.tile(
                        [KC, CP], F32, name=f"loc_b{b}k{k}", tag="loc", bufs=3
                    )
                    nc.vector.scalar_tensor_tensor(
                        locp,
                        yb,
                        pyc[:, k : k + 1],
                        t1_,
                        op0=mybir.AluOpType.mult,
                        op1=mybir.AluOpType.add,
                    )
                    # range reduction: t (ACT), t+MAGIC (DVE), r - t (DVE),
                    # Sin(-2pi x) on ACT
                    tt_ = wp.tile([KC, CP], F32, name=f"tt_b{b}k{k}", tag="tt", bufs=3)
                    nc.scalar.activation(
                        tt_,
                        locp,
                        mybir.ActivationFunctionType.Copy,
                        scale=1.0 / TWO_PI,
                    )
                    rq_ = wp.tile([KC, CP], F32, name=f"rq_b{b}k{k}", tag="rq", bufs=3)
                    nc.vector.tensor_scalar(
                        rq_,
                        locp,
                        1.0 / TWO_PI,
                        MAGIC,
                        op0=mybir.AluOpType.mult,
                        op1=mybir.AluOpType.add,
                    )
                    dd_ = wp.tile([KC, CP], F32, name=f"dd_b{b}k{k}", tag="dd", bufs=3)
                    nc.vector.scalar_tensor_tensor(
                        dd_,
                        rq_,
                        MAGIC,
                        tt_,
                        op0=mybir.AluOpType.subtract,
                        op1=mybir.AluOpType.subtract,
                    )
                    e = wp.tile(
                        [KC, CP], BF16, name=f"embT_b{b}k{k}", tag=f"embT{k}", bufs=2
                    )
                    nc.scalar.activation(
                        e, dd_, mybir.ActivationFunctionType.Sin, scale=-TWO_PI
                    )
                    embT[(b, k)] = e

                for j, (c0, _) in enumerate(C_CHUNKS):
                    jj = b * len(C_CHUNKS) + j
                    offs = offs_all[0:KC, jj : jj + 1]

                    sc = psmall.tile([KC, OP], F32, name=f"sc_b{b}j{j}", tag="sc")
                    for k in range(len(D_CHUNKS)):
                        nc.tensor.matmul(
                            sc,
                            embT[(b, k)][:, c0 : c0 + KC],
                            headsT_sb[k],
                            start=(k == 0),
                            stop=(k == len(D_CHUNKS) - 1),
                        )
                    ex = wp.tile([KC, OP], BF16, name=f"expT_b{b}j{j}", tag=f"expT{j}")
                    nc.scalar.activation(
                        ex, sc, mybir.ActivationFunctionType.Exp, bias=offs
                    )
                    expT[(b, j)] = ex

            def compute_wB(b):
                """softmax denominators + reciprocals for batch b."""
                sume = psmall.tile(
                    [128, len(O_CHUNKS)], F32, name=f"sume_b{b}", tag="sc"
                )
                for oc, o0 in enumerate(O_CHUNKS):
                    for j in range(len(C_CHUNKS)):
                        nc.tensor.matmul(
                            sume[0:128, oc : oc + 1],
                            expT[(b, j)][:, o0 : o0 + 128],
                            ones_sb,
                            start=(j == 0),
                            stop=(j == len(C_CHUNKS) - 1),
                        )
                for oc in range(len(O_CHUNKS)):
                    iv = wp.tile([128, 1], F32, name=f"inv_b{b}o{oc}", tag=f"inv{oc}")
                    nc.vector.reciprocal(iv, sume[0:128, oc : oc + 1])
                    inv[(b, oc)] = iv

            obt = {}

            def big_tile(b, th):
                """one T tile of the big matmul (meg tiles prefetched).
                Evacuations land in a full-T SBUF tile per O chunk; each
                t-tile's slice is stored as soon as it's evacuated so the
                final drain after the last matmul stays short."""
                # prefetch the next tile's meg while this one computes
                if th + 1 < NT:
                    load_tile(b, th + 1)
                elif b + 1 < BPC:
                    load_tile(b + 1, 0)
                t0 = th * TT
                if th == 0:
                    for oc in range(len(O_CHUNKS)):
                        obt[(b, oc)] = outp.tile(
                            [128, T], BF16, name=f"out_b{b}o{oc}", tag=f"out{oc}"
                        )
                for oc, o0 in enumerate(O_CHUNKS):
                    ob = obt[(b, oc)]
                    pbs = [
                        psbig.tile(
                            [128, MM_N], F32, name=f"pb_b{b}t{th}o{oc}n{nt}", tag="pb"
                        )
                        for nt in range(TT // MM_N)
                    ]
                    for j in range(len(C_CHUNKS)):
                        lhsT = expT[(b, j)][:, o0 : o0 + 128]
                        for nt in range(TT // MM_N):
                            nc.tensor.matmul(
                                pbs[nt],
                                lhsT,
                                megs[(b, th, j)][:, nt * MM_N : (nt + 1) * MM_N],
                                start=(j == 0),
                                stop=(j == len(C_CHUNKS) - 1),
                            )
                    for nt in range(TT // MM_N):
                        dst = ob[:, t0 + nt * MM_N : t0 + (nt + 1) * MM_N]
                        if (oc * 2 + nt) % 8 < 5:
                            nc.vector.tensor_scalar_mul(dst, pbs[nt], inv[(b, oc)])
                        else:
                            nc.scalar.activation(
                                dst,
                                pbs[nt],
                                mybir.ActivationFunctionType.Copy,
                                scale=inv[(b, oc)],
                            )
                for oc, o0 in enumerate(O_CHUNKS):
                    # store this t-tile's slice; last chunk duplicates rows
                    # 142:256, store only 256:270
                    if oc == 2:
                        nc.sync.dma_start(
                            out=out[b, 256:O, t0 : t0 + TT],
                            in_=obt[(b, oc)][256 - O_CHUNKS[2] : 128, t0 : t0 + TT],
                        )
                    else:
                        nc.sync.dma_start(
                            out=out[b, o0 : o0 + 128, t0 : t0 + TT],
                            in_=obt[(b, oc)][:, t0 : t0 + TT],
                        )

            # Schedule: big(0) must only wait for batch 0's weights; the
            # sume matmuls for batch b+1 (which block the in-order PE queue
            # on ACT's exp) are deferred past big(b)'s first t-tile.
            compute_wA(0)
            compute_wB(0)
            load_tile(0, 0)
            for b in range(BPC):
                if b + 1 < BPC:
                    compute_wA(b + 1)
                big_tile(b, 0)
                if b + 1 < BPC:
                    compute_wB(b + 1)
                for th in range(1, NT):
                    big_tile(b, th)
    nc.compile()
    return nc


def _get_program():
    if "nc" not in _CACHE:
        _CACHE["nc"] = _build_program()
    return _CACHE["nc"]


def kernel(meg, positions, heads, invalid_mask, trace=False):
    global LAST_RESULTS
    meg = np.asarray(meg, dtype=np.float32).astype(BF_NP)         # [B, C, T] bf16
    positions = np.asarray(positions, dtype=np.float32)
    heads = np.asarray(heads, dtype=np.float32)

    headsTf = heads.T.astype(BF_NP)                              # [D, O] bf16
    # heads chunks side by side [KC, 3*OP]
    headsT = np.zeros((KC, len(D_CHUNKS) * OP), BF_NP)
    for k, d0 in enumerate(D_CHUNKS):
        headsT[:, k * OP : k * OP + O] = headsTf[d0 : d0 + KC, :]
    p3c = _fourier_consts()                                      # [KPAD, D]
    # per-partition columns per d-chunk: px, py, const down the 96 partitions
    pxq = np.stack([p3c[0, d0 : d0 + KC] for d0 in D_CHUNKS], axis=1)
    pyq = np.stack([p3c[1, d0 : d0 + KC] for d0 in D_CHUNKS], axis=1)
    cnq = np.stack([p3c[2, d0 : d0 + KC] for d0 in D_CHUNKS], axis=1)
    maskf = invalid_mask.astype(np.float32) * np.float32(NEG_BIG)  # [B, C]
    # per-chunk mask rows; overlap-duplicated weight rows forced to "masked"
    maskfp = np.zeros((B, len(C_CHUNKS), KC), np.float32)
    for j, (c0, nz) in enumerate(C_CHUNKS):
        maskfp[:, j, :] = maskf[:, c0 : c0 + KC]
        if nz:
            maskfp[:, j, :nz] = NEG_BIG
    F0COLS = 2 * CP + 3 * len(D_CHUNKS)
    FCOLS = (BPC - 1) * 2 * CP + BPC * len(C_CHUNKS)

    nc = _get_program()
    in_maps = []
    for c in range(NCORES):
        s = slice(c * BPC, (c + 1) * BPC)
        fconst0 = np.zeros((KC, F0COLS), np.float32)
        fconst0[:, 0:C] = positions[c * BPC, :, 0]
        fconst0[:, CP : CP + C] = positions[c * BPC, :, 1]
        fconst0[:, 2 * CP : 2 * CP + 3] = pxq
        fconst0[:, 2 * CP + 3 : 2 * CP + 6] = pyq
        fconst0[:, 2 * CP + 6 : 2 * CP + 9] = cnq
        fconst = np.zeros((KC, FCOLS), np.float32)
        for bb in range(1, BPC):
            o0_ = (bb - 1) * 2 * CP
            fconst[:, o0_ : o0_ + C] = positions[c * BPC + bb, :, 0]
            fconst[:, o0_ + CP : o0_ + CP + C] = positions[c * BPC + bb, :, 1]
        fconst[:, (BPC - 1) * 2 * CP :] = (
            maskfp[s].transpose(2, 0, 1).reshape(KC, BPC * len(C_CHUNKS))
        )
        in_maps.append(
            {
                "meg": np.ascontiguousarray(meg[s]),
                "fconst0": fconst0,
                "fconst": fconst,
                "headsT": headsT,
            }
        )

    res = run_bass_kernel_spmd(nc, in_maps, core_ids=list(range(NCORES)), trace=trace)
    LAST_RESULTS = res
    return np.concatenate([r["out"] for r in res.results], axis=0).astype(np.float32)
